# revision 7
# baseline (speedup 1.0000x reference)
"""Trainium2 Bass kernel for nn_BlockR_86045374808442 (sparse_attention).

Math (reference):
    r  = rmsnorm(x)                       # over EMB
    a  = r @ W1^T ; b = r @ W2^T          # [B,T,H]
    y  = exp(cumlogsumexp(a) + cumlogsumexp(b) - 2 log t)   # causal, per feature
    out = x + rmsnorm(y) @ W3^T

Key identities used:
  * rmsnorm(x) @ W = rms_x[t] * (x @ W): the per-token scalar commutes, so we
    fold rms_x into x on the host (xs = (x * rms_x).T in bf16).
  * cumlogsumexp in linear space: exp(la) = cumsum(exp(a)) -- values stay well
    inside fp32 range for this problem's data distribution.
  * y' = cumsum(exp(a)) * cumsum(exp(b)) = y * t^2.  rmsnorm is scale-invariant
    per token, so the 1/t^2 factor and the second rmsnorm reduce to a per-token
    scalar applied on the host: out = x + s[t] * (y' @ W3^T), with
    s[t] = rsqrt(ssq'[t]/(H t^4) + eps) / t^2,  ssq'[t] = sum_h y'^2.

Sharding: 8 cores = 2 batch-halves x 4 HID-shards (1024 features each).
Each core computes its y' slice fully locally (the scan runs over the full T
on the free axis), producing a partial u = y' @ W3_k^T [T,E] plus ssq' [T].
The host sums the 4 partials per batch and applies x + s[t] * U.

Device layout per core (E=1024, HK=1024, T=4096, chunked by TC=512 tokens):
  xs  [E, T]  bf16   rms-scaled x, transposed (host-prepped)
  w1t,w2t [E, HK] bf16 ; w3t [HK, E] bf16 (host-transposed slices)
  g1[h,t] = sum_e w1t[e,h] xs[e,t]   (PE, k=e, 8x128)
  ea = exp(g1) (ACT, reads PSUM directly) ; same for g2/eb
  ca = scan-add(ea) over t (DVE tensor_tensor_scan, fp32 state, carries
       chained across chunks via initial=prev[:, -1:])
  y' = ca*cb (DVE) ; ysq = y'^2 (ACT)
  ssq += ones^T @ ysq (PE, k=h) ; u[t,e] = sum_h y'[h,t] w3t[h,e] (PE)
  u DMA'd straight from PSUM to DRAM.
"""

import os
import sys
from contextlib import ExitStack

import numpy as np
import ml_dtypes

import bass_rust
import concourse.bass as bass
import concourse.mybir as mybir
import concourse.tile as tile
from concourse.bass_utils import run_bass_kernel_spmd

F32 = mybir.dt.float32
BF16 = mybir.dt.bfloat16

B, T, E, H = 2, 4096, 1024, 4096
NCORES = 8
NB = 2            # batch shards
NH = NCORES // NB  # hid shards
HK = H // NH      # features per core
TC = 512          # token chunk
EPS = 1e-6

_MAX_WAITS = 1  # this walrus build allows a single sync-wait per instruction


def _split_excess_waits(nc):
    """Split instructions carrying >1 semaphore wait into EventSemaphore
    prefix chains (walrus codegen limit on this image)."""
    n_split = 0
    for fn in nc.m.functions:
        for blk in fn.blocks:
            out = []
            for inst in blk.instructions:
                si = getattr(inst, "sync_info", None)
                waits = list(si.on_wait) if (si is not None and si.on_wait) else []
                if len(waits) > _MAX_WAITS:
                    keep = waits[:_MAX_WAITS]
                    extra = waits[_MAX_WAITS:]
                    for i in range(0, len(extra), _MAX_WAITS):
                        chunk = extra[i : i + _MAX_WAITS]
                        out.append(
                            mybir.InstEventSemaphore(
                                name=nc.get_next_instruction_name(),
                                engine=inst.engine,
                                sync_info=bass_rust.SyncInfo(
                                    on_wait=chunk, on_update=[]
                                ),
                            )
                        )
                        n_split += 1
                    si.on_wait = keep
                out.append(inst)
            blk.instructions[:] = out
    return n_split


def build_nc(t=T, tc=TC, e=E, hk=HK):
    ke = e // 128    # e-chunks (matmul k for g)
    kh = hk // 128   # h-chunks (matmul k for u / partitions of y)
    nchunk = t // tc
    mt = tc // 128   # t-subtiles per chunk for the u matmul
    nsz = min(512, e)  # e output column tile size for u
    ne = e // nsz

    nc = bass.Bass()
    xs_d = nc.declare_dram_parameter("xs", [e, t], BF16, isOutput=False)
    w1_d = nc.declare_dram_parameter("w1t", [e, hk], BF16, isOutput=False)
    w2_d = nc.declare_dram_parameter("w2t", [e, hk], BF16, isOutput=False)
    w3_d = nc.declare_dram_parameter("w3t", [hk, e], BF16, isOutput=False)
    u_d = nc.declare_dram_parameter("u", [t, e], F32, isOutput=True)
    ssq_d = nc.declare_dram_parameter("ssq", [1, t], F32, isOutput=True)

    with tile.TileContext(nc) as tc_ctx, ExitStack() as ctx:
        singles = ctx.enter_context(tc_ctx.tile_pool(name="singles", bufs=1))
        work = ctx.enter_context(tc_ctx.tile_pool(name="work", bufs=2))
        gps_pool = ctx.enter_context(
            tc_ctx.tile_pool(name="gps", bufs=3, space="PSUM")
        )
        ups_pool = ctx.enter_context(
            tc_ctx.tile_pool(name="ups", bufs=3, space="PSUM")
        )
        sps_pool = ctx.enter_context(
            tc_ctx.tile_pool(name="sps", bufs=1, space="PSUM")
        )

        w1_sb = singles.tile([128, ke, hk], BF16)
        w2_sb = singles.tile([128, ke, hk], BF16)
        w3_sb = singles.tile([128, kh, e], BF16)
        ones_sb = singles.tile([128, 1], BF16)
        ssq_row = singles.tile([1, t], F32)

        nc.vector.memset(ones_sb, 1.0)
        for kk in range(ke):
            nc.sync.dma_start(
                out=w1_sb[:, kk, :], in_=w1_d[kk * 128 : (kk + 1) * 128, :]
            )
            nc.sync.dma_start(
                out=w2_sb[:, kk, :], in_=w2_d[kk * 128 : (kk + 1) * 128, :]
            )
        for kk in range(kh):
            nc.sync.dma_start(
                out=w3_sb[:, kk, :], in_=w3_d[kk * 128 : (kk + 1) * 128, :]
            )

        prev_ca = None
        prev_cb = None
        for ci in range(nchunk):
            tsl = slice(ci * tc, (ci + 1) * tc)

            xs_sb = work.tile([128, ke, tc], BF16, tag="xs")
            for kk in range(ke):
                nc.sync.dma_start(
                    out=xs_sb[:, kk, :], in_=xs_d[kk * 128 : (kk + 1) * 128, tsl]
                )

            # g = W^T-slice @ xs, exp straight out of PSUM
            ea_sb = work.tile([128, kh, tc], BF16, tag="ea")
            eb_sb = work.tile([128, kh, tc], BF16, tag="eb")
            for w_sb, e_sb in ((w1_sb, ea_sb), (w2_sb, eb_sb)):
                for m in range(kh):
                    gps = gps_pool.tile([128, tc], F32, tag="g")
                    for kk in range(ke):
                        nc.tensor.matmul(
                            out=gps,
                            lhsT=w_sb[:, kk, m * 128 : (m + 1) * 128],
                            rhs=xs_sb[:, kk, :],
                            start=(kk == 0),
                            stop=(kk == ke - 1),
                        )
                    nc.scalar.activation(
                        out=e_sb[:, m, :],
                        in_=gps,
                        func=mybir.ActivationFunctionType.Exp,
                    )

            # causal cumulative sum of exp along t (per feature row).
            # bf16 outputs (fp32 state within the scan); the cross-chunk carry
            # goes through bf16, same rounding level as the exp inputs.
            ca_sb = work.tile([128, kh, tc], BF16, tag="ca")
            cb_sb = work.tile([128, kh, tc], BF16, tag="cb")
            for e_sb, c_sb, prev in (
                (ea_sb, ca_sb, prev_ca),
                (eb_sb, cb_sb, prev_cb),
            ):
                for kk in range(kh):
                    init = 0.0 if prev is None else prev[:, kk, tc - 1 : tc]
                    nc.vector.tensor_tensor_scan(
                        out=c_sb[:, kk, :],
                        data0=e_sb[:, kk, :],
                        data1=e_sb[:, kk, :],
                        initial=init,
                        op0=mybir.AluOpType.add,
                        op1=mybir.AluOpType.bypass,
                    )
            prev_ca, prev_cb = ca_sb, cb_sb

            # y' = ca*cb (bf16 for the PE), ysq = y'^2
            y_sb = work.tile([128, kh, tc], BF16, tag="y")
            ysq_sb = work.tile([128, kh, tc], BF16, tag="ysq")
            for kk in range(kh):
                nc.vector.tensor_mul(
                    y_sb[:, kk, :], ca_sb[:, kk, :], cb_sb[:, kk, :]
                )
                nc.scalar.square(ysq_sb[:, kk, :], y_sb[:, kk, :])

            # ssq'[t] = sum_h y'^2 : ones^T @ ysq accumulated over h-chunks
            sps = sps_pool.tile([1, tc], F32, tag="s")
            for kk in range(kh):
                nc.tensor.matmul(
                    out=sps,
                    lhsT=ones_sb,
                    rhs=ysq_sb[:, kk, :],
                    start=(kk == 0),
                    stop=(kk == kh - 1),
                )
            nc.scalar.copy(ssq_row[:, tsl], sps)

            # u[t,e] = sum_h y'[h,t] w3t[h,e]; DMA straight from PSUM
            for m in range(mt):
                for nn in range(ne):
                    ups = ups_pool.tile([128, nsz], F32, tag="u")
                    for kk in range(kh):
                        nc.tensor.matmul(
                            out=ups,
                            lhsT=y_sb[:, kk, m * 128 : (m + 1) * 128],
                            rhs=w3_sb[:, kk, nn * nsz : (nn + 1) * nsz],
                            start=(kk == 0),
                            stop=(kk == kh - 1),
                        )
                    u_sb = work.tile([128, nsz], F32, tag="usb")
                    nc.scalar.copy(u_sb, ups)
                    nc.sync.dma_start(
                        out=u_d[
                            ci * tc + m * 128 : ci * tc + (m + 1) * 128,
                            nn * nsz : (nn + 1) * nsz,
                        ],
                        in_=u_sb,
                    )

        nc.sync.dma_start(out=ssq_d[:, :], in_=ssq_row)

    return nc


_NC_CACHE = {}


def _get_nc():
    if "nc" not in _NC_CACHE:
        nc = build_nc()
        _split_excess_waits(nc)
        _NC_CACHE["nc"] = nc
    return _NC_CACHE["nc"]


def _prep_inputs(x, W1, W2, W3):
    """Host-side shard prep. Returns in_maps for the 8 cores."""
    bf16 = ml_dtypes.bfloat16
    rms = 1.0 / np.sqrt((x.astype(np.float64) ** 2).mean(axis=-1) + EPS)  # [B,T]
    xsc = (x.astype(np.float64) * rms[:, :, None]).astype(np.float32)  # [B,T,E]
    xs_b = [np.ascontiguousarray(xsc[b].T).astype(bf16) for b in range(B)]

    w1t = np.ascontiguousarray(W1.T).astype(bf16)  # [E,H]
    w2t = np.ascontiguousarray(W2.T).astype(bf16)  # [E,H]
    w3t = np.ascontiguousarray(W3.T).astype(bf16)  # [H,E]

    in_maps = []
    for c in range(NCORES):
        b, k = divmod(c, NH)
        hsl = slice(k * HK, (k + 1) * HK)
        in_maps.append(
            {
                "xs": xs_b[b],
                "w1t": np.ascontiguousarray(w1t[:, hsl]),
                "w2t": np.ascontiguousarray(w2t[:, hsl]),
                "w3t": np.ascontiguousarray(w3t[hsl, :]),
            }
        )
    return in_maps


def _assemble(x, results):
    """Host-side unshard: out = x + s[t] * sum_k u_k."""
    out = np.empty_like(x)
    tt = np.arange(1, T + 1, dtype=np.float64)
    t2 = tt * tt
    for b in range(B):
        U = results[b * NH]["u"].astype(np.float64)
        S = results[b * NH]["ssq"][0].astype(np.float64)
        for k in range(1, NH):
            U += results[b * NH + k]["u"]
            S += results[b * NH + k]["ssq"][0]
        s = 1.0 / (np.sqrt(S / (H * t2 * t2) + EPS) * t2)  # [T]
        out[b] = x[b] + (U * s[:, None]).astype(np.float32)
    return out


def kernel(x, W1, W2, W3):
    x = np.asarray(x, dtype=np.float32)
    nc = _get_nc()
    in_maps = _prep_inputs(x, np.asarray(W1), np.asarray(W2), np.asarray(W3))
    res = run_bass_kernel_spmd(nc, in_maps, list(range(NCORES)))
    return _assemble(x, res.results)


if __name__ == "__main__":
    # quick self-check with random data against a numpy reference
    rng = np.random.default_rng(0)
    x = rng.standard_normal((B, T, E)).astype(np.float32)
    W1 = (0.02 * rng.standard_normal((H, E))).astype(np.float32)
    W2 = (0.02 * rng.standard_normal((H, E))).astype(np.float32)
    W3 = (0.02 / np.sqrt(24) * rng.standard_normal((E, H))).astype(np.float32)
    out = kernel(x, W1, W2, W3)
    print("out", out.shape, out.dtype)


# revision 20
# speedup vs baseline: 1.0794x; 1.0794x over previous
"""Trainium2 Bass kernel for nn_BlockR_86045374808442 (sparse_attention).

Math (reference):
    r  = rmsnorm(x)                       # over EMB
    a  = r @ W1^T ; b = r @ W2^T          # [B,T,H]
    y  = exp(cumlogsumexp(a) + cumlogsumexp(b) - 2 log t)   # causal, per feature
    out = x + rmsnorm(y) @ W3^T

Key identities used:
  * rmsnorm(x) @ W = rms_x[t] * (x @ W): the per-token scalar commutes, so we
    fold rms_x into x on the host (xs = (x * rms_x).T in bf16).
  * cumlogsumexp in linear space: exp(la) = cumsum(exp(a)) -- values stay well
    inside fp32 range for this problem's data distribution.
  * y' = cumsum(exp(a)) * cumsum(exp(b)) = y * t^2.  rmsnorm is scale-invariant
    per token, so the 1/t^2 factor and the second rmsnorm reduce to a per-token
    scalar applied on the host: out = x + s[t] * (y' @ W3^T), with
    s[t] = rsqrt(ssq'[t]/(H t^4) + eps) / t^2,  ssq'[t] = sum_h y'^2.

Sharding: 8 cores = 2 batch-halves x 4 HID-shards (1024 features each).
Each core computes its y' slice fully locally (the scan runs over the full T
on the free axis), producing a partial u = y' @ W3_k^T [T,E] plus ssq' [T].
The host sums the 4 partials per batch and applies x + s[t] * U.

Device layout per core (E=1024, HK=1024, T=4096, chunked by TC=512 tokens):
  xs  [E, T]  bf16   rms-scaled x, transposed (host-prepped)
  w1t,w2t [E, HK] bf16 ; w3t [HK, E] bf16 (host-transposed slices)
  g1[h,t] = sum_e w1t[e,h] xs[e,t]   (PE, k=e, 8x128)
  ea = exp(g1) (ACT, reads PSUM directly) ; same for g2/eb
  ca = scan-add(ea) over t (DVE tensor_tensor_scan, fp32 state, carries
       chained across chunks via initial=prev[:, -1:])
  y' = ca*cb (DVE) ; ysq = y'^2 (ACT)
  ssq += ones^T @ ysq (PE, k=h) ; u[t,e] = sum_h y'[h,t] w3t[h,e] (PE)
  u DMA'd straight from PSUM to DRAM.
"""

from contextlib import ExitStack

import numpy as np
import ml_dtypes

import bass_rust
import concourse.bass as bass
import concourse.mybir as mybir
import concourse.tile as tile
from concourse.bass_utils import run_bass_kernel_spmd

F32 = mybir.dt.float32
BF16 = mybir.dt.bfloat16

B, T, E, H = 2, 4096, 1024, 4096
NCORES = 8
NB = 2            # batch shards
NH = NCORES // NB  # hid shards
HK = H // NH      # features per core
TC = 512          # token chunk
EPS = 1e-6

_MAX_WAITS = 1  # this walrus build allows a single sync-wait per instruction


def _split_excess_waits(nc):
    """Split instructions carrying >1 semaphore wait into EventSemaphore
    prefix chains (walrus codegen limit on this image)."""
    n_split = 0
    for fn in nc.m.functions:
        for blk in fn.blocks:
            out = []
            for inst in blk.instructions:
                si = getattr(inst, "sync_info", None)
                waits = list(si.on_wait) if (si is not None and si.on_wait) else []
                if len(waits) > _MAX_WAITS:
                    keep = waits[:_MAX_WAITS]
                    extra = waits[_MAX_WAITS:]
                    for i in range(0, len(extra), _MAX_WAITS):
                        chunk = extra[i : i + _MAX_WAITS]
                        out.append(
                            mybir.InstEventSemaphore(
                                name=nc.get_next_instruction_name(),
                                engine=inst.engine,
                                sync_info=bass_rust.SyncInfo(
                                    on_wait=chunk, on_update=[]
                                ),
                            )
                        )
                        n_split += 1
                    si.on_wait = keep
                out.append(inst)
            blk.instructions[:] = out
    return n_split


def build_nc(t=T, tc=TC, e=E, hk=HK, reps=1):
    ke = e // 128    # e-chunks (matmul k for g)
    kh = hk // 128   # h-chunks (matmul k for u / partitions of y)
    nchunk = t // tc
    mt = tc // 128   # t-subtiles per chunk for the u matmul
    nsz = min(512, e)  # e output column tile size for u
    ne = e // nsz

    nc = bass.Bass()
    xs_d = nc.declare_dram_parameter("xs", [e, t], BF16, isOutput=False)
    w1_d = nc.declare_dram_parameter("w1t", [e, hk], BF16, isOutput=False)
    w2_d = nc.declare_dram_parameter("w2t", [e, hk], BF16, isOutput=False)
    w3_d = nc.declare_dram_parameter("w3t", [hk, e], BF16, isOutput=False)
    u_d = nc.declare_dram_parameter("u", [t, e], F32, isOutput=True)
    ssq_d = nc.declare_dram_parameter("ssq", [1, t], F32, isOutput=True)

    with tile.TileContext(nc) as tc_ctx, ExitStack() as ctx:
        singles = ctx.enter_context(tc_ctx.tile_pool(name="singles", bufs=1))
        work = ctx.enter_context(tc_ctx.tile_pool(name="work", bufs=2))
        gps_pool = ctx.enter_context(
            tc_ctx.tile_pool(name="gps", bufs=4, space="PSUM")
        )
        ups_pool = ctx.enter_context(
            tc_ctx.tile_pool(name="ups", bufs=3, space="PSUM")
        )
        sps_pool = ctx.enter_context(
            tc_ctx.tile_pool(name="sps", bufs=1, space="PSUM")
        )

        # per-k-chunk tiles throughout: Tile tracks dependencies per tile, so
        # fine-grained tiles let consumers start as soon as their slice lands.
        w1_sb = [singles.tile([128, hk], BF16, tag=f"w1_{kk}", name=f"w1_{kk}") for kk in range(ke)]
        ones_sb = singles.tile([128, 1], BF16)
        ssq_row = singles.tile([1, t], F32)

        nc.vector.memset(ones_sb, 1.0)

        xs_view = xs_d[:, :].rearrange("(kk p) t -> p kk t", p=128)

        def load_xs(ci):
            xt = work.tile([128, ke, tc], BF16, tag="xs", name=f"xs_{ci}")
            nc.sync.dma_start(
                out=xt, in_=xs_view[:, :, ci * tc : (ci + 1) * tc]
            )
            return [xt[:, kk, :] for kk in range(ke)]

        # first xs chunk (SP queues) + w1 (ACT queues) land in parallel,
        # lowest-kk first, so the first g-matmul accumulation starts ASAP
        xs0_sb = load_xs(0)
        for kk in range(ke):
            nc.sync.dma_start(
                out=w1_sb[kk], in_=w1_d[kk * 128 : (kk + 1) * 128, :]
            )
        w2_view = w2_d[:, :].rearrange("(kk p) h -> p kk h", p=128)
        w3_view = w3_d[:, :].rearrange("(kk p) h -> p kk h", p=128)
        w2_all = singles.tile([128, ke, hk], BF16, name="w2_all")
        w3_all = singles.tile([128, kh, e], BF16, name="w3_all")
        nc.scalar.dma_start(out=w2_all, in_=w2_view)
        nc.scalar.dma_start(out=w3_all, in_=w3_view)
        w2_sb = [w2_all[:, kk, :] for kk in range(ke)]
        w3_sb = [w3_all[:, kk, :] for kk in range(kh)]

        prev_ca = prev_cb = None
        for ci in [c for _ in range(reps) for c in range(nchunk)]:
            tsl = slice(ci * tc, (ci + 1) * tc)

            if ci == 0:
                prev_ca = prev_cb = None

            if ci == 0 and xs0_sb is not None:
                xs_sb = xs0_sb
                xs0_sb = None
            else:
                xs_sb = load_xs(ci)

            # g = W^T-slice @ xs, exp straight out of PSUM; then the causal
            # cumulative sum of exp along t (DVE scan, fp32 state, bf16 out,
            # carry chained across chunks).  g1/g2 interleaved per m-tile so
            # the DVE chain for each h-tile starts as soon as possible.
            ea_sb = [work.tile([128, tc], BF16, tag=f"ea{m}", name=f"ea{m}") for m in range(kh)]
            eb_sb = [work.tile([128, tc], BF16, tag=f"eb{m}", name=f"eb{m}") for m in range(kh)]
            ca_sb = [work.tile([128, tc], BF16, tag=f"ca{m}", name=f"ca{m}") for m in range(kh)]
            cb_sb = [work.tile([128, tc], BF16, tag=f"cb{m}", name=f"cb{m}") for m in range(kh)]
            y_sb = [work.tile([128, tc], BF16, tag=f"y{m}", name=f"y{m}") for m in range(kh)]
            ysq_sb = [work.tile([128, tc], BF16, tag=f"ysq{m}", name=f"ysq{m}") for m in range(kh)]

            for m in range(kh):
                for w_sb, e_sb, c_sb, prev in (
                    (w1_sb, ea_sb, ca_sb, prev_ca),
                    (w2_sb, eb_sb, cb_sb, prev_cb),
                ):
                    gps = gps_pool.tile([128, tc], F32, tag="g")
                    for kk in range(ke):
                        nc.tensor.matmul(
                            out=gps,
                            lhsT=w_sb[kk][:, m * 128 : (m + 1) * 128],
                            rhs=xs_sb[kk],
                            start=(kk == 0),
                            stop=(kk == ke - 1),
                        )
                    nc.scalar.activation(
                        out=e_sb[m],
                        in_=gps,
                        func=mybir.ActivationFunctionType.Exp,
                    )
                    init = 0.0 if prev is None else prev[m][:, tc - 1 : tc]
                    nc.vector.tensor_tensor_scan(
                        out=c_sb[m],
                        data0=e_sb[m],
                        data1=e_sb[m],
                        initial=init,
                        op0=mybir.AluOpType.add,
                        op1=mybir.AluOpType.bypass,
                    )
                # y' = ca*cb (bf16 for the PE), ysq = y'^2
                nc.vector.tensor_mul(y_sb[m], ca_sb[m], cb_sb[m])
                nc.scalar.square(ysq_sb[m], y_sb[m])
            prev_ca, prev_cb = ca_sb, cb_sb

            # ssq'[t] = sum_h y'^2: GpSimd (idle engine) chain-adds the h-chunk
            # tiles, then a single ones-matmul folds the 128 partitions.
            yacc = work.tile([128, tc], BF16, tag="yacc")
            nc.gpsimd.tensor_add(yacc, ysq_sb[0], ysq_sb[1])
            for kk in range(2, kh):
                nc.gpsimd.tensor_add(yacc, yacc, ysq_sb[kk])
            sps = sps_pool.tile([1, tc], F32, tag="s")
            nc.tensor.matmul(
                out=sps, lhsT=ones_sb, rhs=yacc, start=True, stop=True
            )
            nc.scalar.copy(ssq_row[:, tsl], sps)

            # u[t,e] = sum_h y'[h,t] w3t[h,e]
            for m in range(mt):
                for nn in range(ne):
                    ups = ups_pool.tile([128, nsz], F32, tag="u")
                    for kk in range(kh):
                        nc.tensor.matmul(
                            out=ups,
                            lhsT=y_sb[kk][:, m * 128 : (m + 1) * 128],
                            rhs=w3_sb[kk][:, nn * nsz : (nn + 1) * nsz],
                            start=(kk == 0),
                            stop=(kk == kh - 1),
                        )
                    u_sb = work.tile([128, nsz], F32, tag="usb")
                    nc.scalar.copy(u_sb, ups)
                    nc.sync.dma_start(
                        out=u_d[
                            ci * tc + m * 128 : ci * tc + (m + 1) * 128,
                            nn * nsz : (nn + 1) * nsz,
                        ],
                        in_=u_sb,
                    )

        nc.sync.dma_start(out=ssq_d[:, :], in_=ssq_row)

    return nc


_NC_CACHE = {}


def _get_nc():
    if "nc" not in _NC_CACHE:
        nc = build_nc()
        _split_excess_waits(nc)
        _NC_CACHE["nc"] = nc
    return _NC_CACHE["nc"]


def _prep_inputs(x, W1, W2, W3):
    """Host-side shard prep. Returns in_maps for the 8 cores."""
    bf16 = ml_dtypes.bfloat16
    rms = 1.0 / np.sqrt((x.astype(np.float64) ** 2).mean(axis=-1) + EPS)  # [B,T]
    xsc = (x.astype(np.float64) * rms[:, :, None]).astype(np.float32)  # [B,T,E]
    xs_b = [np.ascontiguousarray(xsc[b].T).astype(bf16) for b in range(B)]

    w1t = np.ascontiguousarray(W1.T).astype(bf16)  # [E,H]
    w2t = np.ascontiguousarray(W2.T).astype(bf16)  # [E,H]
    w3t = np.ascontiguousarray(W3.T).astype(bf16)  # [H,E]

    in_maps = []
    for c in range(NCORES):
        b, k = divmod(c, NH)
        hsl = slice(k * HK, (k + 1) * HK)
        in_maps.append(
            {
                "xs": xs_b[b],
                "w1t": np.ascontiguousarray(w1t[:, hsl]),
                "w2t": np.ascontiguousarray(w2t[:, hsl]),
                "w3t": np.ascontiguousarray(w3t[hsl, :]),
            }
        )
    return in_maps


def _assemble(x, results):
    """Host-side unshard: out = x + s[t] * sum_k u_k."""
    out = np.empty_like(x)
    tt = np.arange(1, T + 1, dtype=np.float64)
    t2 = tt * tt
    for b in range(B):
        U = results[b * NH]["u"].astype(np.float64)
        S = results[b * NH]["ssq"][0].astype(np.float64)
        for k in range(1, NH):
            U += results[b * NH + k]["u"]
            S += results[b * NH + k]["ssq"][0]
        s = 1.0 / (np.sqrt(S / (H * t2 * t2) + EPS) * t2)  # [T]
        out[b] = x[b] + (U * s[:, None]).astype(np.float32)
    return out


def kernel(x, W1, W2, W3):
    x = np.asarray(x, dtype=np.float32)
    nc = _get_nc()
    in_maps = _prep_inputs(x, np.asarray(W1), np.asarray(W2), np.asarray(W3))
    res = run_bass_kernel_spmd(nc, in_maps, list(range(NCORES)))
    return _assemble(x, res.results)


if __name__ == "__main__":
    # quick self-check with random data against a numpy reference
    rng = np.random.default_rng(0)
    x = rng.standard_normal((B, T, E)).astype(np.float32)
    W1 = (0.02 * rng.standard_normal((H, E))).astype(np.float32)
    W2 = (0.02 * rng.standard_normal((H, E))).astype(np.float32)
    W3 = (0.02 / np.sqrt(24) * rng.standard_normal((E, H))).astype(np.float32)
    out = kernel(x, W1, W2, W3)
    print("out", out.shape, out.dtype)
